# revision 32
# baseline (speedup 1.0000x reference)
"""Blended-expert MLP (MoE routing) Trainium2 Bass kernel.

Math: reference computes, per layer,
    h = elu( einsum("bi,bio->bo", x, einsum("be,eio->bio", c, w)) + c @ b )
which factorizes as
    h = elu( sum_e (c[:,e] * x) @ W_e  +  c @ b )
(row-scaling commutes with the matmul), so per layer we scale X^T by
c_e on the vector engine (8 small ops) and run 8 [rows,512]x[512,512]
matmuls plus one tiny K=8 matmul for the blended bias, ALL accumulating
into a single PSUM tile. Then ELU, then a PE transpose to produce the
next layer's stationary operand.

Sharding: data-parallel over the batch. B=512 rows split across 8
NeuronCores (64 rows each); the small expert weights (~24 MB fp32) are
replicated to every core. No collectives.

Layout per core:
  stationary operand = (c_e * X)^T chunks [128(i), 64(b)]
  moving operand     = W chunks  [128(i), 4096(e,o)] sliced per expert
  psum out           = [64(b), 512(o)], fp32, accumulated over e and k

Walrus constraint: fused instructions (fp32 matmul with embedded weight
load, DVE scalar_tensor_tensor) can carry at most ONE sync wait.
Structure keeps every instruction at <=1 wait:
  - all host-side constants ship in ONE packed DMA (one semaphore);
  - the bias matmul opens each layer's accumulation group and carries a
    manual dependency on the last DVE x-scaling op, bridging the DVE
    stream onto PE so expert matmuls only wait on their weight DMA;
  - pool buf counts are sized so no tile is reused while a cross-engine
    reader could still be pending.
"""

import numpy as np

B, E, D = 512, 8, 512
NCORES = 8
ROWS = B // NCORES  # 64
KC = D // 128  # 4 contraction chunks of 128

# pack tensor column layout (per 128 partitions)
PK_XT = 0  # [128, 256]: layer-1 x^T chunk k at cols [64k, 64k+64)
PK_CB = 256  # [128, 2048]: c broadcast; col 256e+64k+b = C[b,e], all partitions
PK_BI = PK_CB + E * KC * ROWS  # [8, 1536]: biases, partitions 0..7
PK_ID = PK_BI + 3 * D  # [64, 64]: identity, partitions 0..63
PK_CT = PK_ID + ROWS  # [8, 64]: coef^T, partitions 0..7
PCK = PK_CT + ROWS

# matmul operand dtype: "f32" (exact, 4 cyc/row), "f32r" (fast fp32 mode,
# 1 cyc/row at N>=256), "bf16" (halves weight DMA, 1 cyc/row).
# bf16 is the only mode whose full weight set fits in SBUF without tile
# reuse, which is required to keep every DMA within walrus's
# one-sync-wait-per-instruction limit.
MODE = "bf16"

_NC_CACHE = {}


def _mmdt(mybir, mode):
    return {
        "f32": mybir.dt.float32,
        "f32r": mybir.dt.float32r,
        "bf16": mybir.dt.bfloat16,
    }[mode]


def _build(mode):
    from contextlib import ExitStack

    import concourse.bacc as bacc
    import concourse.mybir as mybir
    import concourse.tile as tile

    f32 = mybir.dt.float32
    mmdt = _mmdt(mybir, mode)
    Alu = mybir.AluOpType
    Act = mybir.ActivationFunctionType

    # Bacc (not raw Bass): its compile() legalizes the TRN2 one-sync-wait-
    # per-instruction limit by splitting excess waits into EventSemaphores
    nc = bacc.Bacc()
    pack_d = nc.declare_dram_parameter("pack", [128, PCK], mmdt, isOutput=False)
    w_d = nc.declare_dram_parameter("w", [3, D, E * D], mmdt, isOutput=False)
    out_d = nc.declare_dram_parameter("out", [ROWS, D], f32, isOutput=True)

    with ExitStack() as ctx:
        tc = ctx.enter_context(tile.TileContext(nc))
        const = ctx.enter_context(tc.tile_pool(name="const", bufs=1))
        # bf16: all 12 weight chunks fit in SBUF (96 KB/partition) with zero
        # slot reuse; fp32 modes must stream with reuse (Bacc legalizes the
        # extra waits)
        wpool = ctx.enter_context(
            tc.tile_pool(name="wp", bufs=12 if mode == "bf16" else 8)
        )
        spool = ctx.enter_context(tc.tile_pool(name="sp", bufs=24))
        hpool = ctx.enter_context(tc.tile_pool(name="hp", bufs=2))
        xpool = ctx.enter_context(tc.tile_pool(name="xp", bufs=2))
        acc_ps = ctx.enter_context(tc.tile_pool(name="acc", bufs=3, space="PSUM"))
        pt_ps = ctx.enter_context(tc.tile_pool(name="pt", bufs=4, space="PSUM"))
        dmy_ps = ctx.enter_context(tc.tile_pool(name="dmy", bufs=1, space="PSUM"))

        pack_t = const.tile([128, PCK], mmdt)
        nc.sync.dma_start(pack_t[:], pack_d[:])

        coeft_ap = pack_t[0:E, PK_CT : PK_CT + ROWS]
        ident_ap = pack_t[0:ROWS, PK_ID : PK_ID + ROWS]

        xt_ap = pack_t[:, PK_XT : PK_XT + KC * ROWS]  # [128, 256]
        # single dummy-output tile, written once per layer (never read);
        # allocating once avoids tile-release WAW waits between layers
        dmy = dmy_ps.tile([ROWS, ROWS], mmdt, tag="dmy")

        for layer in range(3):
            wts = []
            for k in range(KC):
                # bufs=12 -> all 12 chunks resident, zero slot reuse, so each
                # DMA carries at most its HWDGE lane-predecessor wait
                wt = wpool.tile([128, E * D], mmdt, tag="w")
                nc.sync.dma_start(wt[:], w_d[layer, 128 * k : 128 * (k + 1), :])
                wts.append(wt)

            # scale x^T by c_e along the batch (free) dim: one DVE op per
            # expert over all 4 chunks at once
            scaled = []
            last_tt = None
            for e in range(E):
                sc = spool.tile([128, KC * ROWS], mmdt, tag="sc")
                last_tt = nc.vector.tensor_tensor(
                    out=sc[:],
                    in0=xt_ap,
                    in1=pack_t[:, PK_CB + KC * ROWS * e : PK_CB + KC * ROWS * (e + 1)],
                    op=Alu.mult,
                )
                scaled.append(sc)

            # dummy transpose: a 1-wait PE instruction that absorbs the
            # dependency on this layer's stationary-source producer (pack
            # DMA for layer 1, ACT evacuations later), so the bias matmul
            # below only needs its DVE bridge wait (walrus allows one sync
            # wait per fused-weight-load matmul)
            dummy_mm = nc.tensor.transpose(dmy[:], xt_ap[0:ROWS, 0:ROWS], ident_ap)

            # one accumulation group: bias matmul (K=8) + 32 expert matmuls
            acc = acc_ps.tile([ROWS, D], f32, tag="acc")
            bias_mm = nc.tensor.matmul(
                acc[:],
                coeft_ap,
                pack_t[0:E, PK_BI + D * layer : PK_BI + D * (layer + 1)],
                start=True,
                stop=False,
            )
            # bridge: PE inherits the whole DVE scaling stream (and,
            # transitively, the ACT evacuations the scaling waited on)
            tile.add_dep_helper(bias_mm.ins, last_tt.ins, True, "dve->pe bridge")
            tile.add_dep_helper(bias_mm.ins, dummy_mm.ins, True, "order after dummy")
            for e in range(E):
                for k in range(KC):
                    nc.tensor.matmul(
                        acc[:],
                        scaled[e][:, ROWS * k : ROWS * (k + 1)],
                        wts[k][:, D * e : D * (e + 1)],
                        start=False,
                        stop=(e == E - 1 and k == KC - 1),
                    )

            if layer < 2:
                # elu(x) = max(x,0) + min(exp(x)-1, 0)
                ex = hpool.tile([ROWS, D], f32, tag="ex")
                nc.scalar.activation(ex[:], acc[:], Act.Exp)
                nc.vector.tensor_scalar(
                    ex[:], ex[:], 1.0, 0.0, Alu.subtract, Alu.min
                )
                h = hpool.tile([ROWS, D], mmdt, tag="h")
                nc.vector.scalar_tensor_tensor(
                    out=h[:],
                    in0=acc[:],
                    scalar=0.0,
                    in1=ex[:],
                    op0=Alu.max,
                    op1=Alu.add,
                )
                # transpose h [64,512] -> next stationary [128, 4*64]
                xt_t = xpool.tile([128, KC * ROWS], mmdt, tag="xt")
                for k in range(KC):
                    pt = pt_ps.tile([128, ROWS], mmdt, tag="pt")
                    nc.tensor.transpose(
                        pt[:], h[:, 128 * k : 128 * (k + 1)], ident_ap
                    )
                    nc.scalar.copy(xt_t[:, ROWS * k : ROWS * (k + 1)], pt[:])
                xt_ap = xt_t[:]
            else:
                out_t = hpool.tile([ROWS, D], f32, tag="out")
                nc.scalar.copy(out_t[:], acc[:])
                nc.gpsimd.dma_start(out_d[:], out_t[:])

    nc.compile()
    return nc


def _get_nc(mode):
    if mode not in _NC_CACHE:
        _NC_CACHE[mode] = _build(mode)
    return _NC_CACHE[mode]


def _prep_in_maps(inputs, mode):
    import ml_dtypes

    X = np.asarray(inputs["X"], np.float32)
    C = np.asarray(inputs["blending_coef"], np.float32)
    ws = [np.asarray(inputs[f"w_l{i}"], np.float32) for i in (1, 2, 3)]
    bs = [np.asarray(inputs[f"b_l{i}"], np.float32) for i in (1, 2, 3)]

    mm_np = {"f32": np.float32, "f32r": np.float32, "bf16": ml_dtypes.bfloat16}[mode]

    # W[l][i, e*D+o] = w_l[e, i, o]
    W = np.stack([w.transpose(1, 0, 2).reshape(D, E * D) for w in ws]).astype(mm_np)

    in_maps = []
    for c in range(NCORES):
        rs = slice(c * ROWS, (c + 1) * ROWS)
        pack = np.zeros((128, PCK), np.float32)
        # xt chunks: pack[p, 64k+b] = X[rows][b, 128k+p]
        xt = np.ascontiguousarray(X[rs].T)  # [512, 64]
        pack[:, PK_XT : PK_XT + KC * ROWS] = (
            xt.reshape(KC, 128, ROWS).transpose(1, 0, 2).reshape(128, KC * ROWS)
        )
        # c broadcast: pack[p, PK_CB + 256e + 64k + b] = C[rs][b, e]
        cb = np.broadcast_to(
            C[rs].T[:, None, :], (E, KC, ROWS)
        ).reshape(1, E * KC * ROWS)
        pack[:, PK_CB : PK_CB + E * KC * ROWS] = cb
        for li, b in enumerate(bs):
            pack[0:E, PK_BI + D * li : PK_BI + D * (li + 1)] = b
        pack[0:ROWS, PK_ID : PK_ID + ROWS] = np.eye(ROWS, dtype=np.float32)
        pack[0:E, PK_CT : PK_CT + ROWS] = C[rs].T
        in_maps.append({"pack": pack.astype(mm_np), "w": W})
    return in_maps


def run(inputs, mode=MODE, trace=False):
    """Returns (output [512,512] fp32, BassKernelResults)."""
    from concourse.bass_utils import run_bass_kernel_spmd

    nc = _get_nc(mode)
    in_maps = _prep_in_maps(inputs, mode)
    res = run_bass_kernel_spmd(nc, in_maps, list(range(NCORES)), trace=trace)
    out = np.concatenate([r["out"] for r in res.results], axis=0)
    return out, res


def kernel(**inputs) -> np.ndarray:
    out, _ = run(inputs)
    return out
